# revision 20
# baseline (speedup 1.0000x reference)
"""Birth-death loss kernel v6 for 8 TRN2 NeuronCores.

Per core (2 batches): endpoints are fetched with chunked dma_gather
(256-byte blocks of 64 f32, the hardware minimum; the gather ucode caps
num_idxs at 1024 per call).  Selection of the wanted element from each
block is split across three engines so no engine exceeds the DMA
transfer rate (1.46us per 1024-endpoint chunk):

  - PE builds D = w - iota into an alternating PSUM bank with two
    accumulating broadcast matmuls (identity x w_bc, minus-ones-row x
    iota_bc), ~0.9us/chunk on an otherwise idle engine.
  - DVE fuses mask+multiply into one scalar_tensor_tensor
    ((0 is_equal D) mult V) and does the per-block reduce: 2 ops,
    ~1.25us/chunk.
  - Pool (GPSIMD) generates gather descriptors (~1.38us/chunk) and
    finishes the scalar reduction at the end.

Known HW limits found while tuning (CoreSim/TimelineSim accept all of
these; the device does not): dma_gather num_idxs > 1024 and
elem_size < 64 f32 crash the exec unit; InstTensorTensorReduce and
gpsimd is_equal fail at runtime/codegen; walrus lowers indirect_dma
offsets as one-descriptor-per-partition rows only.

Host prep ships the block index (k>>6, int16, 16-wrapped x8 replicated,
the dma_gather format) and the in-block offset (j&63 as bf16, packed) -
pure index layout, no float math on host.

Endpoint order per stream (g=batch-in-core, t=interval tensor):
  k = e*32768 + c*8192 + n   (e: 0 birth / 1 death, c: class, n: interval)
"""

import numpy as np

import concourse.bass as bass
import concourse.bacc as bacc
import concourse.mybir as mybir
from concourse import library_config
from concourse.bass_utils import run_bass_kernel_spmd

B, C, H, W, N = 16, 4, 512, 512, 8192
NCORES = 8
BS = B // NCORES               # 2 batches/core
PRED_SZ = BS * C * H * W       # 2097152
G0 = (1, 1, 2, 1)
G1 = (0, 1, 0, 2)
NGOOD = BS * (sum(G0) + sum(G1))

NSTREAM = 4                    # (g, t) pairs: (0,0),(0,1),(1,0),(1,1)
KS = C * N * 2                 # endpoints per stream = 65536
CHUNK = 1024                   # endpoints per dma_gather call (ucode max)
NCH = KS // CHUNK              # 64 chunks per stream
NTOT = NSTREAM * NCH           # 256 chunks
GPC = CHUNK // 128             # 8 block-columns per chunk
VB = 6                         # gather buffers
WB = 2                         # VM buffers

f32 = mybir.dt.float32
bf16 = mybir.dt.bfloat16
i16 = mybir.dt.int16
Alu = mybir.AluOpType
X = mybir.AxisListType.X

STREAMS = [(g, t) for g in range(BS) for t in range(2)]
CNT = {0: G0, 1: G1}
HALF = KS // 256               # 256 sel cols per stream half

# load order: blk0a, blk0b, iota, moh, iden, w0, blk1, w1, blk2, w2, blk3, w3
BLK_RDY = [32, 112, 144, 176]
W_RDY = [96, 128, 160, 192]


def _build_nc():
    nc = bacc.Bacc(
        "TRN2", target_bir_lowering=False, debug=False, num_devices=NCORES,
        dynamic_dma_scratch_size=3 * 2**15, detect_race_conditions=False,
    )

    pred = nc.dram_tensor("pred", [PRED_SZ // 64, 64], f32, kind="ExternalInput").ap()
    d_blk = [nc.dram_tensor(f"blk{s}", [128, KS // 16], i16, kind="ExternalInput").ap()
             for s in range(NSTREAM)]
    d_w = [nc.dram_tensor(f"w{s}", [128, KS // 128], bf16, kind="ExternalInput").ap()
           for s in range(NSTREAM)]
    d_iota = nc.dram_tensor("iotaf", [1, 64], bf16, kind="ExternalInput").ap()
    d_iden = nc.dram_tensor("iden", [128, 128], bf16, kind="ExternalInput").ap()
    d_moh = nc.dram_tensor("moh", [1, 128], bf16, kind="ExternalInput").ap()
    outd = nc.dram_tensor("out", [1, 1], f32, kind="ExternalOutput").ap()

    sb_blk = [nc.alloc_sbuf_tensor(f"sb_blk{s}", [128, KS // 16], i16).ap()
              for s in range(NSTREAM)]
    sb_w = [nc.alloc_sbuf_tensor(f"sb_w{s}", [128, KS // 128], bf16).ap()
            for s in range(NSTREAM)]
    sb_iota = nc.alloc_sbuf_tensor("sb_iota", [1, 64], bf16).ap()
    sb_iden = nc.alloc_sbuf_tensor("sb_iden", [128, 128], bf16).ap()
    sb_moh = nc.alloc_sbuf_tensor("sb_moh", [1, 128], bf16).ap()
    sb_V = [nc.alloc_sbuf_tensor(f"sb_V{v}", [128, GPC * 64], f32).ap()
            for v in range(VB)]
    sb_VM = [nc.alloc_sbuf_tensor(f"sb_VM{v}", [128, GPC * 64], f32).ap()
             for v in range(WB)]
    sb_sel = [nc.alloc_sbuf_tensor(f"sb_sel{s}", [128, KS // 128], f32).ap()
              for s in range(NSTREAM)]
    sb_d = [nc.alloc_sbuf_tensor(f"sb_d{s}", [128, HALF], f32).ap()
            for s in range(NSTREAM)]
    sb_sq = nc.alloc_sbuf_tensor("sb_sq", [128, HALF], f32).ap()
    sb_part = nc.alloc_sbuf_tensor("sb_part", [128, 32], f32).ap()
    sb_res = nc.alloc_sbuf_tensor("sb_res", [1, 1], f32).ap()
    ps_D = [nc.alloc_psum_tensor(f"ps_D{v}", [128, GPC * 64], f32).ap()
            for v in range(2)]

    with (
        nc.Block() as block,
        nc.semaphore("dma_in") as dma_in,
        nc.semaphore("gat") as gat,
        nc.semaphore("ped") as ped,        # PE: D bank ready
        nc.semaphore("sttd") as sttd,      # DVE STT done: V free, D bank free
        nc.semaphore("vt") as vt,          # tail-op drain chain
        nc.semaphore("v_done") as v_done,
    ):

        @block.sync
        def _(sy):
            nch0 = CHUNK // 16
            sy.dma_start(out=sb_blk[0][:, 0:nch0], in_=d_blk[0][:, 0:nch0]
                         ).then_inc(dma_in, 16)
            sy.dma_start(out=sb_blk[0][:, nch0:], in_=d_blk[0][:, nch0:]
                         ).then_inc(dma_in, 16)
            sy.dma_start(out=sb_iota, in_=d_iota).then_inc(dma_in, 16)
            sy.dma_start(out=sb_moh, in_=d_moh).then_inc(dma_in, 16)
            sy.dma_start(out=sb_iden, in_=d_iden).then_inc(dma_in, 16)
            sy.dma_start(out=sb_w[0], in_=d_w[0]).then_inc(dma_in, 16)
            for s in range(1, NSTREAM):
                sy.dma_start(out=sb_blk[s], in_=d_blk[s]).then_inc(dma_in, 16)
                sy.dma_start(out=sb_w[s], in_=d_w[s]).then_inc(dma_in, 16)
            sy.wait_ge(v_done, 2)
            sy.dma_start(out=outd, in_=sb_res).then_inc(dma_in, 16)

        @block.gpsimd
        def _(g):
            g.load_library(library_config.mlp)
            nidx_reg = g.alloc_register("nidx")
            g.reg_mov(nidx_reg, CHUNK)
            for cg in range(NTOT):
                s, c = divmod(cg, NCH)
                grp = STREAMS[s][0]
                if cg == 0:
                    g.wait_ge(dma_in, 16)
                elif c == 0 or cg == 1:
                    g.wait_ge(dma_in, BLK_RDY[s])
                if cg >= VB:
                    g.wait_ge(sttd, cg - VB + 1)
                src = pred[grp * (PRED_SZ // 128):(grp + 1) * (PRED_SZ // 128), :]
                g.dma_gather(
                    out_ap=sb_V[cg % VB].rearrange("p (n e) -> p n e", e=64),
                    in_ap=src,
                    idxs_ap=sb_blk[s][:, c * (CHUNK // 16):(c + 1) * (CHUNK // 16)],
                    num_idxs=CHUNK,
                    num_idxs_reg=nidx_reg,
                    elem_size=64,
                ).then_inc(gat, 16)
            g.wait_ge(v_done, 1)
            g.tensor_reduce(sb_res, sb_part, axis=mybir.AxisListType.XYZWC,
                            op=Alu.add)
            g.tensor_scalar(sb_res, sb_res, float(NGOOD), None, Alu.add
                            ).then_inc(v_done, 1)

        @block.tensor
        def _(te):
            iota_bc = sb_iota.rearrange("o (g e) -> o g e", g=1).broadcast_to(
                [1, GPC, 64]
            )
            te.wait_ge(dma_in, W_RDY[0])
            for cg in range(NTOT):
                s, c = divmod(cg, NCH)
                if c == 0 and cg > 0:
                    te.wait_ge(dma_in, W_RDY[s])
                if cg >= 2:
                    te.wait_ge(sttd, cg - 1)
                w_bc = sb_w[s][:, c * GPC:(c + 1) * GPC].unsqueeze(-1
                    ).broadcast_to([128, GPC, 64])
                psv = ps_D[cg % 2].rearrange("p (g e) -> p g e", e=64)
                te.matmul(psv, sb_iden, w_bc, start=True, stop=False)
                te.matmul(psv, sb_moh, iota_bc, start=False, stop=True
                          ).then_inc(ped, 1)

        @block.vector
        def _(v):
            tc = [0]

            def T(ins):
                tc[0] += 1
                ins.then_inc(vt, 1)
                v.wait_ge(vt, tc[0])
                return ins

            def reduce_for(k):
                s2, c2 = divmod(k, NCH)
                vmv = sb_VM[k % WB].rearrange("p (n e) -> p n e", e=64)
                v.tensor_reduce(
                    sb_sel[s2][:, c2 * GPC:(c2 + 1) * GPC], vmv, axis=X,
                    op=Alu.add,
                )

            HH = HALF // 2

            def tail_for(s2, half=None):
                if half is None:
                    lo, hi = 0, HALF
                else:
                    lo, hi = half * HH, (half + 1) * HH
                T(v.tensor_tensor(
                    sb_d[s2][:, lo:hi], sb_sel[s2][:, lo:hi],
                    sb_sel[s2][:, HALF + lo:HALF + hi], Alu.subtract,
                ))

            def tail2_for(s2, half=None, pcol=None):
                if half is None:
                    lo, hi, c4s = 0, HALF, range(C)
                else:
                    lo, hi = half * HH, (half + 1) * HH
                    c4s = range(2 * half, 2 * half + 2)
                if pcol is None:
                    pcol = s2
                T(v.tensor_tensor(
                    sb_sq[:, lo:hi], sb_d[s2][:, lo:hi], sb_d[s2][:, lo:hi],
                    Alu.mult,
                ))
                T(v.tensor_reduce(
                    sb_part[:, pcol:pcol + 1], sb_sq[:, lo:hi], axis=X,
                    op=Alu.add,
                ))
                t = STREAMS[s2][1]
                for c4 in c4s:
                    cnt = CNT[t][c4]
                    if cnt == 0:
                        continue
                    dsl = sb_d[s2][0:cnt, 64 * c4:64 * c4 + 1]
                    v.scalar_tensor_tensor(
                        sb_part[0:cnt, 4 + 4 * s2 + c4:5 + 4 * s2 + c4], dsl,
                        -2.0, dsl, Alu.mult, Alu.mult,
                    )

            v.memset(sb_part, 0.0)
            for cg in range(NTOT):
                buf = sb_V[cg % VB].rearrange("p (n e) -> p n e", e=64)
                vmv = sb_VM[cg % WB].rearrange("p (n e) -> p n e", e=64)
                dv = ps_D[cg % 2].rearrange("p (g e) -> p g e", e=64)
                v.wait_ge(gat, 16 * (cg + 1))
                v.wait_ge(ped, cg + 1)
                # VM = (0 == D) * V   (fused mask+select)
                v.scalar_tensor_tensor(
                    vmv, dv, 0.0, buf, Alu.is_equal, Alu.mult
                ).then_inc(sttd, 1)
                if cg >= 1:
                    reduce_for(cg - 1)     # gap op for the STT
                for s2 in range(NSTREAM - 1):
                    if cg == (s2 + 1) * NCH + 2:
                        tail_for(s2)
                    elif cg == (s2 + 1) * NCH + 4:
                        tail2_for(s2)
                # stream 3 first half: its sel columns land at cg 239
                if cg == 3 * NCH + 50:
                    tail_for(3, half=0)
                elif cg == 3 * NCH + 52:
                    tail2_for(3, half=0, pcol=3)
            reduce_for(NTOT - 1)
            tail_for(3, half=1)
            tail2_for(3, half=1, pcol=20)
            T(v.memset(sb_sq[0:1, 0:1], 0.0))   # drain corrections
            v.nop().then_inc(v_done, 1)

    nc.compile()
    return nc


_NC = None


def _get_nc():
    global _NC
    if _NC is None:
        _NC = _build_nc()
    return _NC


def _host_prep(iv):
    """iv: (C, N, 2, 2) int32 for one (group, tensor) stream.
    Returns (blk16 [128, KS//16] int16 16-wrapped x8, w [128, KS//128] bf16)."""
    import ml_dtypes
    i = iv[:, :, :, 0].astype(np.int32)   # (C, N, 2)
    j = iv[:, :, :, 1].astype(np.int32)
    i2 = i + 512 * np.arange(C, dtype=np.int32)[:, None, None]
    # k-order: (e, c, n)
    i2k = np.transpose(i2, (2, 0, 1)).reshape(KS)
    jk = np.transpose(j, (2, 0, 1)).reshape(KS)
    blk = ((i2k << 3) | (jk >> 6)).astype(np.int16)
    blk16 = np.tile(blk.reshape(KS // 16, 16).T, (8, 1))
    w = (jk & 63).astype(ml_dtypes.bfloat16).reshape(KS // 128, 128).T.copy()
    return blk16, w


def make_in_maps(prediction, intervals_comp_0, intervals_comp_1):
    import ml_dtypes
    iotaf = np.arange(64, dtype=ml_dtypes.bfloat16).reshape(1, 64)
    iden = np.eye(128, dtype=ml_dtypes.bfloat16)
    moh = np.full((1, 128), -1, dtype=ml_dtypes.bfloat16)
    ivs = {0: intervals_comp_0, 1: intervals_comp_1}
    in_maps = []
    for m in range(NCORES):
        sl = slice(m * BS, (m + 1) * BS)
        predc = np.ascontiguousarray(prediction[sl], dtype=np.float32).reshape(
            PRED_SZ // 64, 64
        )
        im = {"pred": predc, "iotaf": iotaf, "iden": iden, "moh": moh}
        for s, (g, t) in enumerate(STREAMS):
            blk16, w = _host_prep(np.asarray(ivs[t][sl][g]))
            im[f"blk{s}"] = blk16
            im[f"w{s}"] = w
        in_maps.append(im)
    return in_maps


def kernel(prediction, intervals_comp_0, intervals_comp_1, **run_kwargs):
    nc = _get_nc()
    in_maps = make_in_maps(prediction, intervals_comp_0, intervals_comp_1)
    res = run_bass_kernel_spmd(nc, in_maps, list(range(NCORES)), **run_kwargs)
    total = np.float32(0.0)
    for r in res.results:
        total += np.float32(r["out"].reshape(())[()])
    kernel.last_result = res
    return np.array(total, dtype=np.float32)
